# revision 12
# baseline (speedup 1.0000x reference)
"""Trainium2 Bass kernel for nn_CustomLoss (CrossEntropy + binary-remap BCE).

loss = mean_i[ logsumexp(pred_i) - pred_i[t_i] ]
     + 100 * mean_i[ 1{ LUT[argmax(pred_i)] != LUT[t_i] } ]

with LUT = [0,0,1,1,1,1,1,1,0,0]  (LUT[j] = 1 iff 2 <= j <= 7).

Sharding: data-parallel over the batch axis across 8 NeuronCores.  The host
CLASS-BUCKETS the rows: all rows with target class PERM[b] land in bucket b
(cols [200b, 200b+200)) of the per-core [128, 10, 2000] class-major layout,
with classes reordered as PERM = [2..7, 0, 1, 8, 9] so that

  * mid-6 classes = rows 0:6, outer-4 = rows 6:10 (contiguous max trees),
  * bucket b's target logit is row b, so the 10 x_t bucket sums collapse to
    ONE strided diagonal access per tile (rows 2i,2i+1 x their col blocks),
  * bt = LUT[target] is 1 on cols [0,1200) and 0 on [1200,2000), so the
    mismatch count is one fused is_gt+accumulate per half.

Two-stage DVE structure to amortize per-instruction overhead: per 400-col
tile only the wide 2-input ops run (m1/o1 max pairs, l1 = E lo+hi halves),
writing into full-width [128, k, 2000] buffers; the narrow follow-ups
(m2/m6/m4, l2/l3/s, mismatch STT) then run once per 1200/800-col bt-half.
Everything streams bf16; exp and the two ln+accum run on ACT.  Final fold
on the host from one [128, 9] f32 result.  All-zero pad rows are exact:
ln adds ln(10) each, x_t and mismatch add 0.
"""

import numpy as np

# ---------------------------------------------------------------- constants
N = 2_000_000
C = 10
N_CORES = 8
P = 128
BUCKET_COLS = 200                 # per-class bucket width (cols per partition)
W_CORE = BUCKET_COLS * C          # 2000
ROWS_CORE_PAD = P * W_CORE        # 256,000 rows per core incl. pads
PERM = [2, 3, 4, 5, 6, 7, 0, 1, 8, 9]   # class of row r / bucket b
TILE_WS = [200, 400, 600, 600, 200]
N_TILES = len(TILE_WS)
HALVES = [(0, 1200, 1), (1200, 2000, 0)]   # (lo, hi, bt)
N_PADS = N_CORES * ROWS_CORE_PAD - N  # 48,000 all-zero pad rows

_CACHE = {}


# ------------------------------------------------------------- device build
def _build_nc():
    import concourse.tile as tile
    from concourse import bacc, mybir

    f32 = mybir.dt.float32
    bf16 = mybir.dt.bfloat16
    A = mybir.ActivationFunctionType
    alu = mybir.AluOpType

    nc = bacc.Bacc("TRN2", target_bir_lowering=False, debug=False,
                   num_devices=N_CORES)
    comb_ds = [
        nc.dram_tensor(f"comb{i}", [P, wi * C], bf16,
                       kind="ExternalInput").ap()
        for i, wi in enumerate(TILE_WS)
    ]
    out_d = nc.dram_tensor("out", [P, 9], f32, kind="ExternalOutput").ap()

    with tile.TileContext(nc) as tc:
        with (
            tc.tile_pool(name="io", bufs=1) as io,
            tc.tile_pool(name="ep", bufs=1) as ep,
            tc.tile_pool(name="wp", bufs=1) as wp,
            tc.tile_pool(name="cp", bufs=1) as cp,
        ):
            m1a = cp.tile([P, 3, W_CORE], bf16)
            o1a = cp.tile([P, 2, W_CORE], bf16)
            l1a = cp.tile([P, 5, W_CORE], bf16)
            s_all = cp.tile([P, W_CORE], bf16)
            # out slots: 0:2 ln accums, 2:7 per-tile x_t sums, 7:9 gt counts
            acc = cp.tile([P, 9], f32)
            tile_lo = np.cumsum([0] + TILE_WS)

            # ---- phase A: all DMAs + exps
            cvs, ets, cts = [], [], []
            for i, w in enumerate(TILE_WS):
                ct = io.tile([P, C * w], bf16, tag=f"comb{i}")
                nc.sync.dma_start(ct[:], comb_ds[i])
                cv = ct[:].rearrange("p (c w) -> p c w", c=C)
                et = ep.tile([P, C, w], bf16, tag=f"E{i}")
                nc.scalar.activation(et[:], cv, A.Exp)
                cts.append(ct)
                cvs.append(cv)
                ets.append(et)

            # ---- stage 1 per tile: wide 2-input ops into full-width bufs
            def stage1(i):
                cv, et = cvs[i], ets[i]
                w = TILE_WS[i]
                lo = int(tile_lo[i])
                nc.vector.tensor_tensor(l1a[:, :, lo:lo + w], et[:, 0:5, :],
                                        et[:, 5:10, :], op=alu.add)
                nc.vector.tensor_tensor(m1a[:, :, lo:lo + w], cv[:, 0:3, :],
                                        cv[:, 3:6, :], op=alu.max)
                nc.vector.tensor_tensor(o1a[:, :, lo:lo + w], cv[:, 6:8, :],
                                        cv[:, 8:10, :], op=alu.max)
                # x_t sums: bucket b's logit is row b; within this tile the
                # (row, col-block) pairs form a diagonal of 200-col blocks
                # with block step w/200 + 1
                nb = w // BUCKET_COLS
                b0 = lo // BUCKET_COLS
                step = nb + 1
                v2 = cts[i][:].rearrange("p (k j) -> p k j", j=BUCKET_COLS)
                k0 = b0 * nb
                bs = wp.tile([P, nb, BUCKET_COLS], bf16, tag=f"bs_{i}")
                nc.vector.tensor_scalar(
                    bs[:], v2[:, k0:k0 + (nb - 1) * step + 1:step, :],
                    0.0, 0.0,
                    op0=alu.add, op1=alu.add, accum_out=acc[:, 2 + i:3 + i])

            # ---- stage 2 per bt-half: narrow follow-ups, once
            def stage2(h):
                lo, hi, bt = HALVES[h]
                hw = hi - lo
                l2 = wp.tile([P, 2, hw], bf16, tag=f"l2_{h}")
                nc.vector.tensor_tensor(l2[:], l1a[:, 0:2, lo:hi],
                                        l1a[:, 2:4, lo:hi], op=alu.add)
                l3 = wp.tile([P, hw], bf16, tag=f"l3_{h}")
                nc.vector.tensor_tensor(l3[:], l2[:, 0, :], l2[:, 1, :],
                                        op=alu.add)
                nc.vector.tensor_tensor(s_all[:, lo:hi], l3[:],
                                        l1a[:, 4, lo:hi], op=alu.add)
                lns = wp.tile([P, hw], bf16, tag=f"lns_{h}")
                nc.scalar.activation(lns[:], s_all[:, lo:hi], A.Ln,
                                     accum_out=acc[:, h:h + 1])
                m2 = wp.tile([P, hw], bf16, tag=f"m2_{h}")
                nc.vector.tensor_tensor(m2[:], m1a[:, 0, lo:hi],
                                        m1a[:, 1, lo:hi], op=alu.max)
                m6 = wp.tile([P, hw], bf16, tag=f"m6_{h}")
                nc.vector.tensor_tensor(m6[:], m2[:], m1a[:, 2, lo:hi],
                                        op=alu.max)
                m4 = wp.tile([P, hw], bf16, tag=f"m4_{h}")
                nc.vector.tensor_tensor(m4[:], o1a[:, 0, lo:hi],
                                        o1a[:, 1, lo:hi], op=alu.max)
                a, b = (m4, m6) if bt == 1 else (m6, m4)
                mo = wp.tile([P, hw], bf16, tag=f"mo_{h}")
                nc.vector.scalar_tensor_tensor(
                    mo[:], a[:], 0.0, b[:], op0=alu.bypass, op1=alu.is_gt,
                    accum_out=acc[:, 7 + h:8 + h])

            stage1(0)
            stage1(1)
            stage1(2)
            stage2(0)
            stage1(3)
            stage1(4)
            stage2(1)

            nc.sync.dma_start(out_d[:], acc[:])

    # Force a single activation table containing both Exp and Ln so the
    # compiler does not ping-pong ACT_TABLE_LOADs.
    import concourse.bacc as bacc_mod
    from concourse.hw_specs import get_activation_tables
    orig = get_activation_tables(nc.m.arch)
    combined = None
    for k, v in orig.items():
        if (mybir.ActivationFunctionType.Exp in v
                and mybir.ActivationFunctionType.Ln in v):
            combined = k
            break
    if combined is not None:
        patched = {k: (v if k == combined else set()) for k, v in orig.items()}
        saved = bacc_mod.get_activation_tables
        bacc_mod.get_activation_tables = lambda arch: patched
        try:
            nc.compile()
        finally:
            bacc_mod.get_activation_tables = saved
    else:
        nc.compile()
    return nc


def _get_nc():
    if "nc" not in _CACHE:
        _CACHE["nc"] = _build_nc()
    return _CACHE["nc"]


# ------------------------------------------------------------------- host
def _host_prep(pred, target):
    """Class-bucketed shard/pack: bf16 tiles [P, 10, 400] per core."""
    import ml_dtypes

    pred = np.asarray(pred)
    if pred.dtype != ml_dtypes.bfloat16:
        pred = pred.astype(np.float32).astype(ml_dtypes.bfloat16)
    pred = pred[:, PERM]              # class perm: row r holds class PERM[r]
    target = np.asarray(target).astype(np.int32)
    # bucket index of each row: inverse perm of its target class
    inv = np.empty(C, np.int64)
    inv[np.asarray(PERM)] = np.arange(C)
    tb = inv[target]

    order = np.argsort(tb, kind="stable")
    counts = np.bincount(tb, minlength=C)
    offs = np.zeros(C + 1, np.int64)
    offs[1:] = np.cumsum(counts)

    in_maps = []
    for k in range(N_CORES):
        R = np.full((C, BUCKET_COLS * P), -1, np.int64)
        for b in range(C):
            cnt = int(counts[b])
            base, rem = divmod(cnt, N_CORES)
            share = base + (1 if k < rem else 0)
            assert share <= BUCKET_COLS * P, (
                f"bucket {b} overflow on core {k}: {share}")
            start = offs[b] + k * base + min(k, rem)
            R[b, :share] = order[start:start + share]
        # [C, P*200] -> [C, P, 200] -> [P, C, 200] -> [P, W_CORE]
        Rpw = R.reshape(C, P, BUCKET_COLS).transpose(1, 0, 2)

        flat = Rpw.reshape(-1)
        Xg = pred[np.where(flat >= 0, flat, 0)]
        Xg[flat < 0] = ml_dtypes.bfloat16(0.0)
        # [P, C_bucket, 200, C_row] -> [P, C_row, C_bucket*200]
        Xc = Xg.reshape(P, C, BUCKET_COLS, C).transpose(0, 3, 1, 2) \
               .reshape(P, C, W_CORE)

        m = {}
        tlo = np.cumsum([0] + TILE_WS)
        for i, wi in enumerate(TILE_WS):
            sl = Xc[:, :, tlo[i]:tlo[i + 1]]
            m[f"comb{i}"] = np.ascontiguousarray(sl).reshape(P, C * wi)
        in_maps.append(m)
    return in_maps


def kernel(pred, target):
    from concourse.bass_utils import run_bass_kernel_spmd

    nc = _get_nc()
    in_maps = _host_prep(pred, target)
    res = run_bass_kernel_spmd(nc, in_maps, core_ids=list(range(N_CORES)))

    s_ln = s_xt = s_gt = 0.0
    for k in range(N_CORES):
        o = res.results[k]["out"].astype(np.float64)
        s_ln += o[:, 0:2].sum()
        s_xt += o[:, 2:7].sum()
        s_gt += o[:, 7:9].sum()

    # all-zero pad rows: s = 10 -> ln(10); x_t and mismatch add exactly 0.
    s_ln -= N_PADS * np.log(10.0)

    ce = (s_ln - s_xt) / N
    bce = 100.0 * s_gt / N
    return np.float32(ce + bce)
